# revision 28
# baseline (speedup 1.0000x reference)
"""AdaConv2d (per-pixel 3x3 dynamic conv) on 8 TRN2 NeuronCores.

out[b,c,h,w] = sum_t x_pad[b,c,h+dh(t),w+dw(t)] * dk[b,c,t,h,w]

Sharding: pure data parallel over batch (B=8 -> one batch element per core).

Per-core layout: partition p = 2c+s (c = channel 0..63, s = H-half 0..1); each
partition holds its unpadded half-plane of x ([64 rows x 128 cols], row f =
x[c, 64s+f]) resident in SBUF while dynamic_kernel streams through in
(row-block, tap) tiles. Tap shifts are free-dim window reads; W-boundary
columns are zeroed in the product tile; H-boundary rows use parity-masked
halo row tiles (the mid-boundary halo lives in the neighbor partition, so it
is DMA'd separately and masked by a per-partition parity vector).

Engines: DVE computes the per-tap products (f32 in, bf16 out); the 9-tap sum
is accumulated on the TensorEngine via identity-matmul into PSUM (f32); ACT
drains PSUM to bf16 SBUF staging; output is stored bf16 and upcast on host.
Memory-bound: ~44 MB/core HBM traffic at ~360-400 GB/s/core.
"""

import numpy as np

from concourse import bacc, bass, tile
from concourse import mybir
from concourse.ap import AP
from concourse.bass_utils import run_bass_kernel_spmd
from concourse.masks import make_identity

F32 = mybir.dt.float32
BF16 = mybir.dt.bfloat16

B, C, H, W = 8, 64, 128, 128
K = 3
NTAP = K * K
NCORES = 8

HALF = H // 2           # 64 rows per half-plane
NBLK = 4                # row-blocks per half
RB = HALF // NBLK       # 16 rows per block
XROWS = HALF + 2        # 66 padded rows per partition
XCOLS = W + 2           # 130 padded cols

_CACHED_NC = None


def _emit(tc, nc, x_ap, dk_ap, pm_dram, out_ap):
    ctx_pools = []

    def pool(name, bufs, space=bass.MemorySpace.SBUF):
        p = tc.tile_pool(name=name, bufs=bufs, space=space)
        ctx_pools.append(p)
        return p.__enter__()

    try:
        const_pool = pool("const", 1)
        x_pool = pool("xp", 1)
        dk_pool = pool("dk", 6)
        tmp_pool = pool("tmp", 4)
        out_pool = pool("osb", 2)
        psum_pool = pool("ps", 8, space=bass.MemorySpace.PSUM)

        identity = const_pool.tile([128, 128], BF16, name="identity")
        make_identity(nc, identity)

        # Partition p = 2c+s (c = channel, s = H-half). x_tile row f holds
        # x[c, 64s+f] (no padding); every DMA spans all 128 partitions in ONE
        # dma_start with outer source dim 64 (c) and contiguous runs, else
        # engine/port splitting craters bandwidth. W-boundary tap columns are
        # memset to zero in tmp; H-boundary tap rows (top of block 0 /
        # bottom of block 1) come from parity-masked halo row tiles.
        pm_ap = const_pool.tile([128, 2], F32, name="pm_ap")
        nc.scalar.dma_start(out=pm_ap[:], in_=pm_dram)
        # halo_bot: even p (s=0) need x row 64, odd don't-care (row 65) -> *even
        # halo_top: odd p (s=1) need x row 63, even don't-care (row 62) -> *odd
        halo_raw = x_pool.tile([128, 2, W], F32, name="halo_raw")
        nc.scalar.dma_start(
            out=halo_raw[:, 0:1, :],
            in_=AP(x_ap.tensor, HALF * W, [[H * W, C], [W, 2], [1, W]]),
        )
        nc.scalar.dma_start(
            out=halo_raw[:, 1:2, :],
            in_=AP(x_ap.tensor, (HALF - 2) * W, [[H * W, C], [W, 2], [1, W]]),
        )
        x_tile = x_pool.tile([128, HALF, W], F32, name="x_tile")
        xsplit = 10
        nc.scalar.dma_start(
            out=x_tile[:, 0:xsplit, :],
            in_=AP(x_ap.tensor, 0, [[H * W, C], [HALF * W, 2], [W, xsplit], [1, W]]),
        )
        nc.scalar.dma_start(
            out=x_tile[:, xsplit:HALF, :],
            in_=AP(
                x_ap.tensor,
                xsplit * W,
                [[H * W, C], [HALF * W, 2], [W, HALF - xsplit], [1, W]],
            ),
        )
        halo = x_pool.tile([128, 2, W], F32, name="halo")
        nc.gpsimd.tensor_scalar_mul(halo[:, 0:1, :], halo_raw[:, 0:1, :], pm_ap[:, 0:1])
        nc.gpsimd.tensor_scalar_mul(halo[:, 1:2, :], halo_raw[:, 1:2, :], pm_ap[:, 1:2])

        blocks = [16, 32, 8, 8]  # small first (fast start), big bulk, short tail
        assert sum(blocks) == HALF
        for b in range(len(blocks)):
            rb = blocks[b]
            r0 = sum(blocks[:b])  # first output row of this block in each half
            ps_tiles = [
                psum_pool.tile([128, 4, 128], F32, name=f"ps_{b}_{j}", tag="ps")
                for j in range(rb * W // 512)
            ]
            for t in range(NTAP):
                dh, dw = t // K - 1, t % K - 1
                dk_t = dk_pool.tile([128, rb, W], F32, name="dk_t", tag="dk")
                dk_src = AP(
                    dk_ap.tensor,
                    t * H * W + r0 * W,
                    [[NTAP * H * W, C], [HALF * W, 2], [W, rb], [1, W]],
                )
                nc.sync.dma_start(out=dk_t[:], in_=dk_src)
                tmp = tmp_pool.tile([128, rb, W], BF16, name="tmp", tag="tmp")
                # valid output columns for this tap; the one boundary column
                # (reading x col -1 or W) contributes zero
                wo = slice(1, W) if dw < 0 else slice(0, W - 1) if dw > 0 else slice(0, W)
                wx = slice(wo.start + dw, wo.stop + dw)
                if dw != 0:
                    zc = slice(0, 1) if dw < 0 else slice(W - 1, W)
                    nc.gpsimd.memset(tmp[:, :, zc], 0.0)
                # rows whose x source row r0+r+dh is inside this half
                er = None  # (tmp row, halo row) needing the halo source
                rlo, rhi = 0, rb
                if b == 0 and dh < 0:
                    rlo, er = 1, (0, 1)  # top edge row: x row -1|63 = halo_top
                elif b == len(blocks) - 1 and dh > 0:
                    rhi, er = rb - 1, (rb - 1, 0)  # bottom: x row 64|128 = halo_bot
                nc.vector.tensor_mul(
                    tmp[:, rlo:rhi, wo],
                    x_tile[:, r0 + dh + rlo : r0 + dh + rhi, wx],
                    dk_t[:, rlo:rhi, wo],
                )
                if er is not None:
                    tr, hr = er
                    nc.gpsimd.tensor_mul(
                        tmp[:, tr : tr + 1, wo],
                        halo[:, hr : hr + 1, wx],
                        dk_t[:, tr : tr + 1, wo],
                    )
                for j in range(len(ps_tiles)):
                    nc.tensor.matmul(
                        ps_tiles[j][:],
                        identity[:],
                        tmp[:, 4 * j : 4 * j + 4, :],
                        start=(t == 0),
                        stop=(t == NTAP - 1),
                    )

            out_sb = out_pool.tile([128, rb, W], BF16, name="out_sb", tag="osb")
            for j in range(len(ps_tiles)):
                nc.scalar.copy(out=out_sb[:, 4 * j : 4 * j + 4, :], in_=ps_tiles[j][:])
            out_dst = AP(
                out_ap.tensor,
                r0 * W,
                [[H * W, C], [HALF * W, 2], [W, rb], [1, W]],
            )
            nc.gpsimd.dma_start(out=out_dst, in_=out_sb[:])
    finally:
        for p in reversed(ctx_pools):
            p.__exit__(None, None, None)


def build_nc():
    global _CACHED_NC
    if _CACHED_NC is not None:
        return _CACHED_NC
    nc = bacc.Bacc("TRN2", target_bir_lowering=False, debug=False, num_devices=NCORES)
    x_ap = nc.dram_tensor("x", [C, H, W], F32, kind="ExternalInput").ap()
    dk_ap = nc.dram_tensor(
        "dynamic_kernel", [C, NTAP, H, W], F32, kind="ExternalInput"
    ).ap()
    pm_dram = nc.dram_tensor("pmask", [128, 2], F32, kind="ExternalInput").ap()
    out_ap = nc.dram_tensor("out", [C, H, W], BF16, kind="ExternalOutput").ap()
    with tile.TileContext(nc) as tc:
        _emit(tc, nc, x_ap, dk_ap, pm_dram, out_ap)
    nc.compile()
    _CACHED_NC = nc
    return nc


def pmask_np() -> np.ndarray:
    p = np.arange(128)
    return np.stack([(p % 2 == 0), (p % 2 == 1)], axis=1).astype(np.float32)


def make_in_maps(x: np.ndarray, dynamic_kernel: np.ndarray, n: int = NCORES):
    pm = pmask_np()
    return [
        {
            "x": np.ascontiguousarray(x[i], dtype=np.float32),
            "dynamic_kernel": np.ascontiguousarray(dynamic_kernel[i], dtype=np.float32),
            "pmask": pm,
        }
        for i in range(n)
    ]


def kernel(x: np.ndarray, dynamic_kernel: np.ndarray) -> np.ndarray:
    nc = build_nc()
    in_maps = make_in_maps(x, dynamic_kernel)
    res = run_bass_kernel_spmd(nc, in_maps, core_ids=list(range(NCORES)))
    out = np.stack([res.results[i]["out"] for i in range(NCORES)], axis=0)
    return out.astype(np.float32)
